# revision 1
# baseline (speedup 1.0000x reference)
"""Trainium2 Bass kernel for nn_H_DYNA_42348377538865 (scatter_memory GRU + memory attention).

Self-contained: shards node dim N=512 across 8 NeuronCores (64 nodes/core),
runs a fully-unrolled 24-step recurrence per core, gathers on host.

Layout: feature-on-partitions, (node, batch) on free dim (col = n_local*32 + b,
NB=2048 cols/core). Key decompositions (validated vs reference in numpy):
  - rolling q-cache: q(h_t) computed once/step; 12 slots in 3x[128,2048] tiles;
    slot j pairs with memory slice s=(j-t)%12 -> 12 precomputed rotation stacks
  - softmax without max-subtraction; fused-mean matmul with M=128 emitting
    [lc;gc] rows 0:64 and replicated sums rows 64:128 so reciprocal is
    broadcast-ready without an extra matmul
  - hypernet nsw = node_emb @ weight_pool precomputed on host (param repack)
  - decode is autoregressive: x_{d+1} = y_d
"""
import numpy as np
import sys

for _p in ("/opt/trn_rl_repo",):
    if _p not in sys.path:
        sys.path.append(_p)

import concourse.bass as bass
import concourse.bacc as bacc
import concourse.mybir as mybir
import concourse.tile as tile
from concourse import bass_utils

B, T, HORIZON, N = 32, 12, 12, 512
IN, OUT, H, P = 1, 1, 64, 32
S, ML, MG, DE = 12, 64, 32, 10
NCORES = 8
NL = N // NCORES        # 64
NB = NL * B             # 2048
NSTEP = T + HORIZON     # 24
CH = 4                  # column chunks
CW = NB // CH           # 512

F32 = mybir.dt.float32
F32R = mybir.dt.float32r
BF16 = mybir.dt.bfloat16
AF = mybir.ActivationFunctionType

CTX_BF16 = True        # nsw + fn in bf16: ctx matmuls 1 cyc/row instead of 4
ADD_ON_GPSIMD = True   # h-update add on gpsimd to unload DVE


def build_nc():
    nc = bacc.Bacc("TRN2", target_bir_lowering=False, debug=False)
    d = {}
    d["xsrc"] = nc.dram_tensor("xsrc", [T, NB], BF16, kind="ExternalInput")
    d["memstack"] = nc.dram_tensor("memstack", [128, S * 3 * 96], BF16, kind="ExternalInput")
    d["nsw"] = nc.dram_tensor("nsw", [64, NL * 64], BF16, kind="ExternalInput")
    d["fmean"] = nc.dram_tensor("fmean", [96, 64], BF16, kind="ExternalInput")
    d["fsum"] = nc.dram_tensor("fsum", [96, 64], BF16, kind="ExternalInput")
    d["zw"] = nc.dram_tensor("zw", [65, 64], BF16, kind="ExternalInput")
    d["rw"] = nc.dram_tensor("rw", [65, 64], BF16, kind="ExternalInput")
    d["cw"] = nc.dram_tensor("cw", [65, 64], BF16, kind="ExternalInput")
    d["qw"] = nc.dram_tensor("qw", [64, 32], BF16, kind="ExternalInput")
    d["ow"] = nc.dram_tensor("ow", [64, 1], BF16, kind="ExternalInput")
    d["bq4"] = nc.dram_tensor("bq4", [128, 1], F32, kind="ExternalInput")
    d["bz"] = nc.dram_tensor("bz", [64, 1], F32, kind="ExternalInput")
    d["br"] = nc.dram_tensor("br", [64, 1], F32, kind="ExternalInput")
    d["bc"] = nc.dram_tensor("bc", [64, 1], F32, kind="ExternalInput")
    d["bo"] = nc.dram_tensor("bo", [1, 1], F32, kind="ExternalInput")
    ys_d = nc.dram_tensor("ys", [HORIZON, NB], BF16, kind="ExternalOutput")

    with tile.TileContext(nc) as tc:
        with (
            tc.tile_pool(name="consts", bufs=1) as cp,
            tc.tile_pool(name="sp", bufs=6) as sp,
            tc.tile_pool(name="pp_lg", bufs=2, space="PSUM") as pp_lg,
            tc.tile_pool(name="pp_fu", bufs=1, space="PSUM") as pp_fu,
            tc.tile_pool(name="pp_acc", bufs=1, space="PSUM") as pp_acc,
            tc.tile_pool(name="pp_z", bufs=1, space="PSUM") as pp_z,
            tc.tile_pool(name="pp_r", bufs=1, space="PSUM") as pp_r,
            tc.tile_pool(name="pp_qp", bufs=1, space="PSUM") as pp_qp,
            tc.tile_pool(name="pp_yp", bufs=1, space="PSUM") as pp_yp,
        ):
            xs = cp.tile([T, NB], BF16)
            nc.sync.dma_start(xs[:], d["xsrc"].ap())
            msk = cp.tile([128, S * 3 * 96], BF16)
            nc.sync.dma_start(msk[:], d["memstack"].ap())
            nsw = cp.tile([64, NL * 64], BF16)
            nc.sync.dma_start(nsw[:], d["nsw"].ap())
            fmean = cp.tile([96, 64], BF16)
            nc.sync.dma_start(fmean[:], d["fmean"].ap())
            fsum = cp.tile([96, 64], BF16)
            nc.sync.dma_start(fsum[:], d["fsum"].ap())
            zw = cp.tile([65, 64], BF16)
            nc.sync.dma_start(zw[:], d["zw"].ap())
            rw = cp.tile([65, 64], BF16)
            nc.sync.dma_start(rw[:], d["rw"].ap())
            cw = cp.tile([65, 64], BF16)
            nc.sync.dma_start(cw[:], d["cw"].ap())
            qw = cp.tile([64, 32], BF16)
            nc.sync.dma_start(qw[:], d["qw"].ap())
            ow = cp.tile([64, 1], BF16)
            nc.sync.dma_start(ow[:], d["ow"].ap())
            bq4 = cp.tile([128, 1], F32)
            nc.sync.dma_start(bq4[:], d["bq4"].ap())
            bz = cp.tile([64, 1], F32)
            nc.sync.dma_start(bz[:], d["bz"].ap())
            br = cp.tile([64, 1], F32)
            nc.sync.dma_start(br[:], d["br"].ap())
            bc = cp.tile([64, 1], F32)
            nc.sync.dma_start(bc[:], d["bc"].ap())
            bo = cp.tile([1, 1], F32)
            nc.sync.dma_start(bo[:], d["bo"].ap())

            qb = []
            for g in range(3):
                q = cp.tile([128, NB], BF16, name=f"qb{g}")
                nc.vector.memset(q[:], 0.0)
                nc.scalar.activation(q[:], q[:], AF.Identity, bias=bq4[:, 0:1])
                qb.append(q)
            hx = cp.tile([65, NB], BF16)
            nc.vector.memset(hx[:], 0.0)
            rhx = cp.tile([65, NB], BF16)
            nc.vector.memset(rhx[:], 0.0)
            ystage = cp.tile([1, NB], BF16)
            nc.sync.dma_start(hx[64:65, :], xs[0:1, :])
            nc.sync.dma_start(rhx[64:65, :], xs[0:1, :])

            for t in range(NSTEP):
                r = t % S
                j = t % S
                g_w, row_w = j // 4, (j % 4) * 32
                for c in range(CH):
                    cs = slice(c * CW, (c + 1) * CW)
                    lg = pp_lg.tile([96, CW], F32, tag="lg")
                    for g in range(3):
                        off = (r * 3 + g) * 96
                        nc.tensor.matmul(
                            lg[:], msk[:, off : off + 96], qb[g][:, cs],
                            start=(g == 0), stop=(g == 2),
                        )
                    ex = sp.tile([96, CW], BF16, tag="ex")
                    nc.scalar.activation(ex[:], lg[:], AF.Exp)
                    fu = pp_fu.tile([64, CW], F32, tag="fu")
                    nc.tensor.matmul(fu[:], fmean[:], ex[:], start=True, stop=True)
                    su = pp_lg.tile([64, CW], F32, tag="lg")
                    nc.tensor.matmul(su[:], fsum[:], ex[:], start=True, stop=True)
                    rt = sp.tile([64, CW], F32, tag="rt")
                    nc.vector.reciprocal_approx_fast(rt[:], su[:])
                    fn = sp.tile([64, CW], BF16, tag="fn")
                    nc.vector.tensor_mul(fn[:], fu[:], rt[:])
                    zp = pp_z.tile([64, CW], F32, tag="zp")
                    nc.tensor.matmul(zp[:], zw[:], hx[:, cs], start=True, stop=True)
                    rp = pp_r.tile([64, CW], F32, tag="rp")
                    nc.tensor.matmul(rp[:], rw[:], hx[:, cs], start=True, stop=True)
                    zt = sp.tile([64, CW], F32, tag="zt")
                    nc.scalar.activation(zt[:], zp[:], AF.Sigmoid, bias=bz[:, 0:1])
                    rs = sp.tile([64, CW], F32, tag="rs")
                    nc.scalar.activation(rs[:], rp[:], AF.Sigmoid, bias=br[:, 0:1])
                    nc.vector.tensor_mul(rhx[0:64, cs], rs[:], hx[0:64, cs])
                    acc = pp_acc.tile([64, CW], F32, tag="acc")
                    nc.tensor.matmul(
                        acc[:], cw[:], rhx[:, cs],
                        start=True, stop=False, skip_group_check=True,
                    )
                    for k in range(16):
                        n = c * 16 + k
                        nsw_ap = nsw[:, n * 64 : (n + 1) * 64]
                        fn_ap = fn[:, k * 32 : (k + 1) * 32]
                        if not CTX_BF16:
                            nsw_ap, fn_ap = nsw_ap, fn_ap
                        nc.tensor.matmul(
                            acc[:, k * 32 : (k + 1) * 32], nsw_ap, fn_ap,
                            start=False, stop=(k == 15), skip_group_check=True,
                        )
                    hc = sp.tile([64, CW], F32, tag="hc")
                    nc.scalar.activation(hc[:], acc[:], AF.Tanh, bias=bc[:, 0:1])
                    dl = sp.tile([64, CW], F32, tag="dl")
                    nc.gpsimd.tensor_sub(dl[:], hc[:], hx[0:64, cs])
                    nc.vector.tensor_mul(dl[:], zt[:], dl[:])
                    add_eng = nc.gpsimd if ADD_ON_GPSIMD else nc.vector
                    add_eng.tensor_add(hx[0:64, cs], hx[0:64, cs], dl[:])
                    # q(h_t) lands directly on the target qbuf slot partitions
                    qp = pp_qp.tile([128, CW], F32, tag="qp")
                    nc.tensor.matmul(
                        qp[row_w : row_w + 32, :], qw[:], hx[0:64, cs],
                        start=True, stop=True, tile_position=(0, row_w),
                    )
                    nc.scalar.activation(
                        qb[g_w][row_w : row_w + 32, cs], qp[row_w : row_w + 32, :],
                        AF.Identity, bias=bq4[row_w : row_w + 32, 0:1],
                    )
                    if t >= T:
                        yp = pp_yp.tile([1, CW], F32, tag="yp")
                        nc.tensor.matmul(yp[:], ow[:], hx[0:64, cs], start=True, stop=True)
                        nc.scalar.activation(ystage[0:1, cs], yp[0:1, :], AF.Identity, bias=bo[0:1, 0:1])
                if t < T - 1:
                    nc.sync.dma_start(hx[64:65, :], xs[t + 1 : t + 2, :])
                    nc.sync.dma_start(rhx[64:65, :], xs[t + 1 : t + 2, :])
                elif t >= T:
                    dstep = t - T
                    nc.sync.dma_start(ys_d[dstep : dstep + 1, :], ystage[0:1, :])
                    if t < NSTEP - 1:
                        nc.sync.dma_start(hx[64:65, :], ystage[0:1, :])
                        nc.sync.dma_start(rhx[64:65, :], ystage[0:1, :])
    nc.compile()
    return nc


def precompute(inp):
    lm = np.asarray(inp["local_mem"], np.float32)
    gm = np.asarray(inp["global_mem"], np.float32)
    Wq = np.asarray(inp["Wq"], np.float32)
    bq = np.asarray(inp["bq"], np.float32)
    node_emb = np.asarray(inp["node_emb"], np.float32)
    wp = np.asarray(inp["weight_pool"], np.float32)
    Wz = np.asarray(inp["Wz"], np.float32)
    bz = np.asarray(inp["bz"], np.float32)
    Wr = np.asarray(inp["Wr"], np.float32)
    br = np.asarray(inp["br"], np.float32)
    Wc = np.asarray(inp["Wc"], np.float32)
    bc = np.asarray(inp["bc"], np.float32)
    Wo = np.asarray(inp["Wo"], np.float32)
    bo = np.asarray(inp["bo"], np.float32)

    c = {}
    c["nsw_full"] = np.einsum("nd,dfh->nfh", node_emb, wp).astype(np.float32)
    memsl = np.concatenate([lm.transpose(2, 0, 1), gm.transpose(2, 0, 1)], axis=1)  # [P,96,S]
    ms = np.zeros((128, S, 3, 96), np.float32)
    for r in range(S):
        for g in range(3):
            for i in range(4):
                s = (4 * g + i - r) % S
                ms[32 * i : 32 * (i + 1), r, g, :] = memsl[:, :, s]
    c["memstack"] = ms.reshape(128, S * 3 * 96)
    lmean, gmean = lm.mean(axis=1), gm.mean(axis=1)
    fs = np.zeros((96, 64), np.float32)
    fs[:ML, :P] = lmean
    fs[ML:, P : 2 * P] = gmean
    c["fmean"] = fs
    fsum = np.zeros((96, 64), np.float32)
    fsum[:ML, :P] = 1.0
    fsum[ML:, P : 2 * P] = 1.0
    c["fsum"] = fsum
    zwm = np.zeros((H + 1, H), np.float32)
    zwm[:H] = Wz[1:]
    zwm[H] = Wz[0]
    c["zw"] = zwm
    rwm = np.zeros((H + 1, H), np.float32)
    rwm[:H] = Wr[1:]
    rwm[H] = Wr[0]
    c["rw"] = rwm
    cc = np.zeros((H + 1, H), np.float32)
    cc[:H] = Wc[1:]
    cc[H] = Wc[0]
    c["cw"] = cc
    c["qw"] = Wq.copy()
    c["ow"] = Wo[:, 0:1].copy()
    c["bq4"] = np.tile(bq, 4).reshape(128, 1)
    c["bz"] = bz.reshape(64, 1)
    c["br"] = br.reshape(64, 1)
    c["bc"] = bc.reshape(64, 1)
    c["bo"] = bo.reshape(1, 1)
    return c


def _bf16(a):
    import ml_dtypes
    return np.ascontiguousarray(a).astype(ml_dtypes.bfloat16)


def make_in_maps(inp):
    c = precompute(inp)
    src = np.asarray(inp["source"], np.float32)
    shared = {
        "memstack": _bf16(c["memstack"]), "fmean": _bf16(c["fmean"]),
        "fsum": _bf16(c["fsum"]), "zw": _bf16(c["zw"]), "rw": _bf16(c["rw"]),
        "cw": _bf16(c["cw"]), "qw": _bf16(c["qw"]), "ow": _bf16(c["ow"]),
        "bq4": c["bq4"], "bz": c["bz"], "br": c["br"], "bc": c["bc"], "bo": c["bo"],
    }
    in_maps = []
    for core in range(NCORES):
        nodes = slice(core * NL, (core + 1) * NL)
        xs = _bf16(src[:, :, nodes, 0].transpose(1, 2, 0).reshape(T, NB))
        nswc = _bf16(c["nsw_full"][nodes].transpose(1, 0, 2).reshape(64, NL * 64))
        in_maps.append(dict(shared, xsrc=xs, nsw=nswc))
    return in_maps


def assemble(results):
    out = np.zeros((B, HORIZON, N, OUT), np.float32)
    for core in range(NCORES):
        nodes = slice(core * NL, (core + 1) * NL)
        ys = np.asarray(results[core]["ys"], np.float32)  # [HORIZON, NB]
        out[:, :, nodes, 0] = ys.reshape(HORIZON, NL, B).transpose(2, 0, 1)
    return out


_NC_CACHE = {}


def kernel(**inputs):
    if "nc" not in _NC_CACHE:
        _NC_CACHE["nc"] = build_nc()
    nc = _NC_CACHE["nc"]
    in_maps = make_in_maps(inputs)
    res = bass_utils.run_bass_kernel_spmd(nc, in_maps, core_ids=list(range(NCORES)))
    return assemble(res.results)



# revision 13
# speedup vs baseline: 1.3638x; 1.3638x over previous
"""Trainium2 Bass kernel for nn_H_DYNA_42348377538865 (scatter_memory GRU + memory attention).

Self-contained: shards node dim N=512 across 8 NeuronCores (64 nodes/core),
runs a fully-unrolled 24-step recurrence per core, gathers on host.

Layout: feature-on-partitions, (node, batch) on free dim (col = n_local*32 + b,
NB=2048 cols/core). Key decompositions (validated vs reference in numpy):
  - rolling q-cache: q(h_t) computed once/step; 12 slots in 3x[128,2048] tiles;
    slot j pairs with memory slice s=(j-t)%12 -> 12 precomputed rotation stacks
  - softmax without max-subtraction; ONE fused matmul [96,128] emits means
    (rows 0:64) and replicated sums (rows 64:128) so the reciprocal broadcasts
  - single act table: sigmoid(x) = 0.5*(1+tanh(x/2)); exp/tanh/copy all live
    in the `exp_and_others` table -> zero table reloads
  - z/r gates in ONE [65,128] matmul + ONE tanh; GRU update via
    scalar_tensor_tensor: 2rh=(1+v)h (0.5 folded into Wc), w=hc-h, s=(1+u)w,
    h+=0.5s
  - q bias bq folded into the exp bias (sum_s bq.mem, constant per mem row)
  - decode: Wo folded into gate weights (no autoregressive y->x loopback);
    y accumulates in a persistent PSUM bank, copied out once at the end
  - hypernet nsw = node_emb @ weight_pool precomputed on host (param repack)
"""
import numpy as np
import sys

for _p in ("/opt/trn_rl_repo",):
    if _p not in sys.path:
        sys.path.append(_p)

import concourse.bass as bass
import concourse.bacc as bacc
import concourse.mybir as mybir
import concourse.tile as tile
from concourse import bass_utils

B, T, HORIZON, N = 32, 12, 12, 512
IN, OUT, H, P = 1, 1, 64, 32
S, ML, MG, DE = 12, 64, 32, 10
NCORES = 8
NL = N // NCORES        # 64
NB = NL * B             # 2048
NSTEP = T + HORIZON     # 24
CH = 4                  # column chunks
CW = NB // CH           # 512

F32 = mybir.dt.float32
BF16 = mybir.dt.bfloat16
AF = mybir.ActivationFunctionType
ALU = mybir.AluOpType


def build_nc():
    nc = bacc.Bacc("TRN2", target_bir_lowering=False, debug=False)
    d = {}
    d["xsrc"] = nc.dram_tensor("xsrc", [T, NB], BF16, kind="ExternalInput")
    d["memstack"] = nc.dram_tensor("memstack", [128, S * 3 * 96], BF16, kind="ExternalInput")
    d["nsw"] = nc.dram_tensor("nsw", [64, NL * 64], BF16, kind="ExternalInput")
    d["fms"] = nc.dram_tensor("fms", [96, 128], BF16, kind="ExternalInput")
    d["zrw"] = nc.dram_tensor("zrw", [65, 128], BF16, kind="ExternalInput")
    d["zrwf"] = nc.dram_tensor("zrwf", [64, 128], BF16, kind="ExternalInput")
    d["cws"] = nc.dram_tensor("cws", [65, 64], BF16, kind="ExternalInput")
    d["cwf"] = nc.dram_tensor("cwf", [64, 64], BF16, kind="ExternalInput")
    d["cwx"] = nc.dram_tensor("cwx", [64, 64], BF16, kind="ExternalInput")
    d["qw"] = nc.dram_tensor("qw", [64, 32], BF16, kind="ExternalInput")
    d["owd"] = nc.dram_tensor("owd", [64, HORIZON * HORIZON], BF16, kind="ExternalInput")
    d["bqlog"] = nc.dram_tensor("bqlog", [96, 1], F32, kind="ExternalInput")
    d["bzr2"] = nc.dram_tensor("bzr2", [128, 1], F32, kind="ExternalInput")
    d["bzrf2"] = nc.dram_tensor("bzrf2", [128, 1], F32, kind="ExternalInput")
    d["bce"] = nc.dram_tensor("bce", [64, 1], F32, kind="ExternalInput")
    d["bcd"] = nc.dram_tensor("bcd", [64, 1], F32, kind="ExternalInput")
    ys_d = nc.dram_tensor("ys", [HORIZON, NB], BF16, kind="ExternalOutput")

    with tile.TileContext(nc) as tc:
        with (
            tc.tile_pool(name="consts", bufs=1) as cp,
            tc.tile_pool(name="sp", bufs=4) as sp,
            tc.tile_pool(name="ps", bufs=2, space="PSUM") as pp,
            tc.tile_pool(name="pq", bufs=1, space="PSUM") as pq,
            tc.tile_pool(name="py", bufs=1, space="PSUM") as py,
        ):
            xs = cp.tile([T, NB], BF16)
            nc.sync.dma_start(xs[:], d["xsrc"].ap())
            msk = cp.tile([128, S * 3 * 96], BF16)
            nc.sync.dma_start(msk[:], d["memstack"].ap())
            nsw = cp.tile([64, NL * 64], BF16)
            nc.sync.dma_start(nsw[:], d["nsw"].ap())
            fms = cp.tile([96, 128], BF16)
            nc.sync.dma_start(fms[:], d["fms"].ap())
            zrw = cp.tile([65, 128], BF16)
            nc.sync.dma_start(zrw[:], d["zrw"].ap())
            zrwf = cp.tile([64, 128], BF16)
            nc.sync.dma_start(zrwf[:], d["zrwf"].ap())
            cws = cp.tile([65, 64], BF16)
            nc.sync.dma_start(cws[:], d["cws"].ap())
            cwf = cp.tile([64, 64], BF16)
            nc.sync.dma_start(cwf[:], d["cwf"].ap())
            cwx = cp.tile([64, 64], BF16)
            nc.sync.dma_start(cwx[:], d["cwx"].ap())
            qw = cp.tile([64, 32], BF16)
            nc.sync.dma_start(qw[:], d["qw"].ap())
            owd = cp.tile([64, HORIZON * HORIZON], BF16)
            nc.sync.dma_start(owd[:], d["owd"].ap())
            bqlog = cp.tile([96, 1], F32)
            nc.sync.dma_start(bqlog[:], d["bqlog"].ap())
            bzr2 = cp.tile([128, 1], F32)
            nc.sync.dma_start(bzr2[:], d["bzr2"].ap())
            bzrf2 = cp.tile([128, 1], F32)
            nc.sync.dma_start(bzrf2[:], d["bzrf2"].ap())
            bce = cp.tile([64, 1], F32)
            nc.sync.dma_start(bce[:], d["bce"].ap())
            bcd = cp.tile([64, 1], F32)
            nc.sync.dma_start(bcd[:], d["bcd"].ap())

            qb = []
            for g in range(3):
                q = cp.tile([128, NB], BF16, name=f"qb{g}")
                nc.vector.memset(q[:], 0.0)
                qb.append(q)
            hx = cp.tile([65, NB], BF16)
            nc.vector.memset(hx[:], 0.0)
            rhx = cp.tile([65, NB], BF16)
            nc.vector.memset(rhx[:], 0.0)
            # y staging: row c*HORIZON + d holds decode step d of chunk c
            ysb = cp.tile([4 * HORIZON, CW], BF16)
            nc.sync.dma_start(hx[64:65, :], xs[0:1, :])
            nc.sync.dma_start(rhx[64:65, :], xs[0:1, :])

            # persistent PSUM: q projections (4 chunks x 32 rows), and the
            # decode y accumulator (chunk c rows 32c:32c+12; each decode step
            # adds Wo^T h into row 32c+d and +0 elsewhere)
            qpb = pq.tile([128, CW], F32)
            ypt = py.tile([128, CW], F32)

            csl = [slice(c * CW, (c + 1) * CW) for c in range(CH)]

            for t in range(NSTEP):
                r = t % S
                j = t % S
                g_w, row_w = j // 4, (j % 4) * 32
                enc = t <= T  # t==12 still uses x-row (x = source[:, -1])

                # --- PE: z|r gate logits ---
                zrp = []
                for c in range(CH):
                    zp = pp.tile([128, CW], F32, tag="fz")
                    if enc:
                        nc.tensor.matmul(zp[:], zrw[:], hx[:, csl[c]],
                                         start=True, stop=True)
                    else:
                        nc.tensor.matmul(zp[:], zrwf[:], hx[0:64, csl[c]],
                                         start=True, stop=True)
                    zrp.append(zp)
                # --- PE: attention logits from q-cache ---
                lgp = []
                for c in range(CH):
                    lg = pp.tile([96, CW], F32, tag="lg")
                    for g in range(3):
                        off = (r * 3 + g) * 96
                        nc.tensor.matmul(
                            lg[:], msk[:, off : off + 96], qb[g][:, csl[c]],
                            start=(g == 0), stop=(g == 2),
                        )
                    lgp.append(lg)
                # --- ACT: gates u|v = tanh((logits+b)/2) ---
                uvl = []
                for c in range(CH):
                    uv = sp.tile([128, CW], BF16, tag="uv", bufs=6)
                    nc.scalar.activation(uv[:], zrp[c][:], AF.Tanh,
                                         bias=(bzr2 if enc else bzrf2)[:, 0:1],
                                         scale=0.5)
                    uvl.append(uv)
                # --- ACT: exp of attention logits ---
                exl = []
                for c in range(CH):
                    ex = sp.tile([96, CW], BF16, tag="ex", bufs=6)
                    nc.scalar.activation(ex[:], lgp[c][:], AF.Exp,
                                         bias=bqlog[:, 0:1])
                    exl.append(ex)
                # --- DVE: 2*r*h = (1+v)*h ---
                for c in range(CH):
                    nc.vector.scalar_tensor_tensor(
                        rhx[0:64, csl[c]], uvl[c][64:128, :], 1.0,
                        hx[0:64, csl[c]], ALU.add, ALU.mult)
                # --- PE: fused mean/sum matmul ---
                fzl = []
                for c in range(CH):
                    fz = pp.tile([128, CW], F32, tag="fz")
                    nc.tensor.matmul(fz[:], fms[:], exl[c][:],
                                     start=True, stop=True)
                    fzl.append(fz)
                # --- DVE: reciprocal of sums; fn = means * recip ---
                rtl = []
                for c in range(CH):
                    rt = sp.tile([64, CW], F32, tag="rt", bufs=5)
                    nc.vector.reciprocal_approx_fast(rt[:], fzl[c][64:128, :])
                    rtl.append(rt)
                fnl = []
                for c in range(CH):
                    fn = sp.tile([64, CW], BF16, tag="fn", bufs=5)
                    nc.vector.tensor_mul(fn[:], fzl[c][0:64, :], rtl[c][:])
                    fnl.append(fn)
                # --- PE: candidate pre-activation: Wc part then hypernet ---
                accl = []
                for c in range(CH):
                    acc = pp.tile([64, CW], F32, tag="acc")
                    if enc:
                        nc.tensor.matmul(acc[:], cws[:], rhx[:, csl[c]],
                                         start=True, stop=False,
                                         skip_group_check=True)
                    else:
                        nc.tensor.matmul(acc[:], cwf[:], rhx[0:64, csl[c]],
                                         start=True, stop=False,
                                         skip_group_check=True)
                        nc.tensor.matmul(acc[:], cwx[:], hx[0:64, csl[c]],
                                         start=False, stop=False,
                                         skip_group_check=True)
                    accl.append(acc)
                for c in range(CH):
                    for k in range(16):
                        n = c * 16 + k
                        nc.tensor.matmul(
                            accl[c][:, k * 32 : (k + 1) * 32],
                            nsw[:, n * 64 : (n + 1) * 64],
                            fnl[c][:, k * 32 : (k + 1) * 32],
                            start=False, stop=(k == 15), skip_group_check=True,
                        )
                # --- ACT: hc = tanh(acc + bc) ---
                hcl = []
                for c in range(CH):
                    hc = sp.tile([64, CW], BF16, tag="hc", bufs=5)
                    nc.scalar.activation(hc[:], accl[c][:], AF.Tanh,
                                         bias=(bce if enc else bcd)[:, 0:1])
                    hcl.append(hc)
                # --- DVE: h += 0.5*(1+u)*(hc-h) ---
                wl = []
                for c in range(CH):
                    w = sp.tile([64, CW], BF16, tag="w", bufs=5)
                    nc.vector.tensor_sub(w[:], hcl[c][:], hx[0:64, csl[c]])
                    wl.append(w)
                sl = []
                for c in range(CH):
                    s2 = sp.tile([64, CW], BF16, tag="s2", bufs=5)
                    nc.vector.scalar_tensor_tensor(
                        s2[:], uvl[c][0:64, :], 1.0, wl[c][:],
                        ALU.add, ALU.mult)
                    sl.append(s2)
                for c in range(CH):
                    nc.vector.scalar_tensor_tensor(
                        hx[0:64, csl[c]], sl[c][:], 0.5, hx[0:64, csl[c]],
                        ALU.mult, ALU.add)
                # --- PE: q projection of new h; decode y projection ---
                if t < NSTEP - 1:
                    for c in range(CH):
                        nc.tensor.matmul(
                            qpb[32 * c : 32 * (c + 1), :], qw[:],
                            hx[0:64, csl[c]], start=True, stop=True,
                            tile_position=(0, 32 * c),
                        )
                if t >= T:
                    dstep = t - T
                    for c in range(CH):
                        nc.tensor.matmul(
                            ypt[32 * c : 32 * c + HORIZON, :],
                            owd[:, HORIZON * dstep : HORIZON * (dstep + 1)],
                            hx[0:64, csl[c]],
                            start=(dstep == 0), stop=(dstep == HORIZON - 1),
                            skip_group_check=True,
                            tile_position=(0, 32 * c),
                        )
                # --- Pool: q-cache slot update ---
                if t < NSTEP - 1:
                    for c in range(CH):
                        nc.gpsimd.tensor_copy(
                            qb[g_w][row_w : row_w + 32, csl[c]],
                            qpb[32 * c : 32 * (c + 1), :])
                # --- DMA: encode x prefetch ---
                if t < T - 1:
                    nc.sync.dma_start(hx[64:65, :], xs[t + 1 : t + 2, :])
                    nc.sync.dma_start(rhx[64:65, :], xs[t + 1 : t + 2, :])

            for c in range(CH):
                nc.gpsimd.tensor_copy(
                    ysb[c * HORIZON : (c + 1) * HORIZON, :],
                    ypt[32 * c : 32 * c + HORIZON, :])
            for c in range(CH):
                nc.sync.dma_start(
                    ys_d.ap()[0:HORIZON, c * CW : (c + 1) * CW],
                    ysb[c * HORIZON : (c + 1) * HORIZON, :])
    nc.compile()
    return nc


def precompute(inp):
    lm = np.asarray(inp["local_mem"], np.float64)
    gm = np.asarray(inp["global_mem"], np.float64)
    Wq = np.asarray(inp["Wq"], np.float64)
    bq = np.asarray(inp["bq"], np.float64)
    node_emb = np.asarray(inp["node_emb"], np.float64)
    wp = np.asarray(inp["weight_pool"], np.float64)
    Wz = np.asarray(inp["Wz"], np.float64)
    bz = np.asarray(inp["bz"], np.float64)
    Wr = np.asarray(inp["Wr"], np.float64)
    br = np.asarray(inp["br"], np.float64)
    Wc = np.asarray(inp["Wc"], np.float64)
    bc = np.asarray(inp["bc"], np.float64)
    Wo = np.asarray(inp["Wo"], np.float64)
    bo = np.asarray(inp["bo"], np.float64)

    c = {}
    c["nsw_full"] = np.einsum("nd,dfh->nfh", node_emb, wp)
    memsl = np.concatenate([lm.transpose(2, 0, 1), gm.transpose(2, 0, 1)], axis=1)  # [P,96,S]
    ms = np.zeros((128, S, 3, 96))
    for rr in range(S):
        for g in range(3):
            for i in range(4):
                s = (4 * g + i - rr) % S
                ms[32 * i : 32 * (i + 1), rr, g, :] = memsl[:, :, s]
    c["memstack"] = ms.reshape(128, S * 3 * 96)
    lmean, gmean = lm.mean(axis=1), gm.mean(axis=1)
    fms = np.zeros((96, 128))
    fms[:ML, :P] = lmean
    fms[ML:, P : 2 * P] = gmean
    fms[:ML, 64 : 64 + P] = 1.0
    fms[ML:, 64 + P : 128] = 1.0
    c["fms"] = fms
    zrw = np.zeros((H + 1, 128))
    zrw[:H, :H] = Wz[1:]
    zrw[H, :H] = Wz[0]
    zrw[:H, H:] = Wr[1:]
    zrw[H, H:] = Wr[0]
    c["zrw"] = zrw
    Wzf = Wz[1:] + Wo @ Wz[0:1]
    Wrf = Wr[1:] + Wo @ Wr[0:1]
    c["zrwf"] = np.concatenate([Wzf, Wrf], axis=1)
    cws = np.zeros((H + 1, H))
    cws[:H] = 0.5 * Wc[1:]
    cws[H] = Wc[0]
    c["cws"] = cws
    c["cwf"] = 0.5 * Wc[1:]
    c["cwx"] = Wo @ Wc[0:1]
    c["qw"] = Wq.copy()
    owd = np.zeros((H, HORIZON * HORIZON))
    for dd in range(HORIZON):
        owd[:, HORIZON * dd + dd] = Wo[:, 0]
    c["owd"] = owd
    c["bqlog"] = np.concatenate([lm.sum(axis=1) @ bq, gm.sum(axis=1) @ bq]).reshape(96, 1)
    c["bzr2"] = (0.5 * np.concatenate([bz, br])).reshape(128, 1)
    c["bzrf2"] = (0.5 * np.concatenate([bz + bo[0] * Wz[0], br + bo[0] * Wr[0]])).reshape(128, 1)
    c["bce"] = bc.reshape(64, 1)
    c["bcd"] = (bc + bo[0] * Wc[0]).reshape(64, 1)
    c["bo"] = float(bo[0])
    return c


def _bf16(a):
    import ml_dtypes
    return np.ascontiguousarray(a).astype(ml_dtypes.bfloat16)


def _f32(a):
    return np.ascontiguousarray(a).astype(np.float32)


def make_in_maps(inp):
    c = precompute(inp)
    src = np.asarray(inp["source"], np.float32)
    shared = {
        "memstack": _bf16(c["memstack"]), "fms": _bf16(c["fms"]),
        "zrw": _bf16(c["zrw"]), "zrwf": _bf16(c["zrwf"]),
        "cws": _bf16(c["cws"]), "cwf": _bf16(c["cwf"]), "cwx": _bf16(c["cwx"]),
        "qw": _bf16(c["qw"]), "owd": _bf16(c["owd"]),
        "bqlog": _f32(c["bqlog"]), "bzr2": _f32(c["bzr2"]),
        "bzrf2": _f32(c["bzrf2"]), "bce": _f32(c["bce"]), "bcd": _f32(c["bcd"]),
    }
    in_maps = []
    for core in range(NCORES):
        nodes = slice(core * NL, (core + 1) * NL)
        xsc = _bf16(src[:, :, nodes, 0].transpose(1, 2, 0).reshape(T, NB))
        nswc = _bf16(c["nsw_full"][nodes].transpose(1, 0, 2).reshape(64, NL * 64))
        in_maps.append(dict(shared, xsrc=xsc, nsw=nswc))
    return in_maps


_BO_CACHE = {}


def assemble(results, bo=0.0):
    out = np.zeros((B, HORIZON, N, OUT), np.float32)
    for core in range(NCORES):
        nodes = slice(core * NL, (core + 1) * NL)
        ys = np.asarray(results[core]["ys"], np.float32) + bo  # [HORIZON, NB]
        out[:, :, nodes, 0] = ys.reshape(HORIZON, NL, B).transpose(2, 0, 1)
    return out


_NC_CACHE = {}


def kernel(**inputs):
    if "nc" not in _NC_CACHE:
        _NC_CACHE["nc"] = build_nc()
    nc = _NC_CACHE["nc"]
    in_maps = make_in_maps(inputs)
    bo = float(np.asarray(inputs["bo"], np.float64)[0])
    res = bass_utils.run_bass_kernel_spmd(nc, in_maps, core_ids=list(range(NCORES)))
    return assemble(res.results, bo)
